# revision 10
# baseline (speedup 1.0000x reference)
"""Trainium2 Bass kernel for nn_Attention_60885456388891 (gnn_message_passing).

Computation (per batch b):
  node_h = h @ W_h2node + b_h2node
  score_n[n] = sum_d tanh(p_node_feats[b,n,d] + node_h[b,d]) * w_alpha1[d]
  node_w = renorm(softmax(score_n) * att_masks)
  node_res_ = sum_n node_w[n] * node_feats[b,n,:]
  (same for relations)
  node_res = glu(cat(node_res_, rela_res_) @ W_ng + b_ng)
  rela_res = glu(cat(rela_res_, node_res) @ W_rg + b_rg)

Strategy (v5): pure data-parallel over batch B=512 across 8 cores.

Key observation: softmax's denominator cancels against the mask renorm
(m*e^s / sum(m*e^s)), so masked nodes/relations are completely irrelevant
to the output.  The host COMPACTS each batch's active nodes/relations
(gather by mask) and pads to Npad/Rpad (runtime-derived from the masks,
rounded up to 32).  Pad positions get pnf = -20*sign(w_alpha) (=> score
-sum|w| => exp ~ e^-8, negligible) and value rows = 0, so no mask tensor
or mask arithmetic exists on device at all.  This cuts the streamed bytes
by ~40% on top of the f16 downcast (~39MB/core vs 56MB).

Per-core pipeline:
  - pnf/prf streamed d-on-partitions [128, BB, KC, Npad]; per-(b,c) DVE
    tensor_scalar_add of the nh/rh bias columns; one big ACT tanh per block.
  - scores: tanh chunk stationary [128d, Npad] x w_alpha chunk column ->
    em columns accumulate in PSUM **in column form** [Npad, G] -- exp (ACT)
    and the 1/sum reduction (ones-column matmul) happen in 128-partition
    form, and the em columns feed the value matmuls directly.
  - values as weights [K=Npad, M=128d] x em column -> X^T columns in the
    k-chunked layout the GLU matmuls consume.
  - GLU biases folded into the matmul: appending the sum-column S with
    rhs=bias_row gives (X@W + S*b) * (1/S) = X@W/S + b exactly.
"""

import numpy as np

import concourse.bass as bass
import concourse.bacc as bacc
import concourse.mybir as mybir
import concourse.tile as tile
from concourse.bass_utils import run_bass_kernel_spmd

# Problem dims (hardcoded per contract)
B, N, R, D = 512, 128, 256, 512
NCORES = 8
BS = B // NCORES          # 64 batches per core
BB = 4                    # batches per stream DMA block
NBLK = BS // BB           # 16 blocks
G = 16                    # batches per group (psum granularity)
GROUPS = BS // G          # 4 groups
NPJ = G // BB             # blocks per group
KC = D // 128             # 4 k-chunks of 128
KC2 = 2 * D // 128        # 8 k-chunks for the 1024-wide GLU matmuls

F32 = mybir.dt.float32
F16 = mybir.dt.float16
AF = mybir.ActivationFunctionType
ALU = mybir.AluOpType


def build_program(Npad, Rpad):
    R0 = min(Rpad, 128)
    R1 = Rpad - R0

    nc = bacc.Bacc("TRN2", target_bir_lowering=False, debug=False)

    def din(name, shape, dt=F16):
        return nc.dram_tensor(name, shape, dt, kind="ExternalInput").ap()

    h_d = din("h", [BS, D])
    pnf_d = din("pnf", [NBLK, 128, BB, KC, Npad])       # d-partition args
    prf_d = din("prf", [NBLK, 128, BB, KC, Rpad])
    nf_d = din("nf", [NBLK, Npad, BB, KC, 128])         # n-partition values
    rf0_d = din("rf0", [NBLK, R0, BB, KC, 128])
    rf1_d = din("rf1", [NBLK, R1, BB, KC, 128]) if R1 else None
    Wn_d = din("w_h2node", [128, KC, D])
    bn_d = din("b_h2node", [128, KC], F32)
    Wr_d = din("w_h2rela", [128, KC, D])
    br_d = din("b_h2rela", [128, KC], F32)
    w1_d = din("w1c", [128, KC])                        # w_alpha1 as columns
    w2_d = din("w2c", [128, KC])
    Wng_d = din("w_ng", [128, KC2, 2, 512])
    Wrg_d = din("w_rg", [128, KC2, 2, 512])
    bngr_d = din("bias_ng", [1, 2, 512])                # bias rows (f16)
    brgr_d = din("bias_rg", [1, 2, 512])
    id_d = din("ident", [128, 128])                     # f16 identity
    ones_d = din("ones_col", [128, 1])                  # f16 ones column

    nres_d = nc.dram_tensor("node_res", [BS, D], F32, kind="ExternalOutput").ap()
    rres_d = nc.dram_tensor("rela_res", [BS, D], F32, kind="ExternalOutput").ap()

    dma = nc.sync.dma_start

    with tile.TileContext(nc) as tc:
        with (
            tc.tile_pool(name="const", bufs=1) as cp,
        ):
            # ---- persistent constants ----
            ident = cp.tile([128, 128], F16)
            dma(out=ident, in_=id_d)
            ones_col = cp.tile([128, 1], F16)
            dma(out=ones_col, in_=ones_d)
            w1c = cp.tile([128, KC], F16)
            dma(out=w1c, in_=w1_d)
            w2c = cp.tile([128, KC], F16)
            dma(out=w2c, in_=w2_d)

            # persistent outputs of phase C / B
            XTn = cp.tile([128, KC, BS], F16, tag="xtn")   # unnormalized Xn^T
            XTr = cp.tile([128, KC, BS], F16, tag="xtr")
            S_sb = cp.tile([1, 2, BS], F32, tag="ssb")     # em column sums
            nhT = cp.tile([128, KC, BS], F32, tag="nht")   # bias columns
            rhT = cp.tile([128, KC, BS], F32, tag="rht")
            # GLU weights/bias rows (DMA'd during the main loop)
            Wng_sb = cp.tile([128, KC2, 2, 512], F16, tag="wng")
            Wrg_sb = cp.tile([128, KC2, 2, 512], F16, tag="wrg")
            bng_row = cp.tile([1, 2, 512], F16, tag="bngr")
            brg_row = cp.tile([1, 2, 512], F16, tag="brgr")

            # ---- prologue: nhT/rhT bias columns = (h @ W + b)^T, computed
            # directly in transposed chunk layout (no row round-trip) ----
            with (
                tc.tile_pool(name="prol", bufs=1) as pp,
                tc.tile_pool(name="prps", bufs=2, space="PSUM") as pps,
            ):
                h_sb = pp.tile([BS, D], F16, tag="h")
                dma(out=h_sb, in_=h_d)
                Wn_sb = pp.tile([128, KC, D], F16, tag="wn")
                dma(out=Wn_sb, in_=Wn_d)
                Wr_sb = pp.tile([128, KC, D], F16, tag="wr")
                dma(out=Wr_sb, in_=Wr_d)
                bn_sb = pp.tile([128, KC], F32, tag="bn")
                dma(out=bn_sb, in_=bn_d)
                br_sb = pp.tile([128, KC], F32, tag="br")
                dma(out=br_sb, in_=br_d)

                hT = pp.tile([128, KC, BS], F16, tag="ht")
                for c in range(KC):
                    tps = pps.tile([128, BS], F16, tag="tps")
                    nc.tensor.transpose(tps, h_sb[:, c * 128:(c + 1) * 128],
                                        ident[:BS, :BS])
                    nc.vector.tensor_copy(hT[:, c, :], tps)
                for W_sb, b_sb, dstT in ((Wn_sb, bn_sb, nhT),
                                         (Wr_sb, br_sb, rhT)):
                    for c in range(KC):
                        ps = pps.tile([128, BS], F32, tag="nhcps")
                        for k in range(KC):
                            nc.tensor.matmul(
                                ps, W_sb[:, k, c * 128:(c + 1) * 128],
                                hT[:, k, :],
                                start=(k == 0), stop=(k == KC - 1))
                        nc.vector.tensor_scalar_add(dstT[:, c, :], ps,
                                                    b_sb[:, c:c + 1])

            # ---- main loop: per-block software pipeline over 16 slots ----
            with (
                tc.tile_pool(name="pnfp", bufs=4) as pnfp,
                tc.tile_pool(name="prfp", bufs=4) as prfp,
                tc.tile_pool(name="nfp", bufs=4) as nfp,
                tc.tile_pool(name="rf0p", bufs=4) as rf0p,
                tc.tile_pool(name="rf1p", bufs=4) as rf1p,
                tc.tile_pool(name="emp", bufs=2) as emp,
                tc.tile_pool(name="em2p", bufs=8) as em2p,
                tc.tile_pool(name="scps", bufs=2, space="PSUM") as scps,
                tc.tile_pool(name="etps", bufs=2, space="PSUM") as etps,
                tc.tile_pool(name="xps", bufs=2, space="PSUM") as xps,
                tc.tile_pool(name="sps", bufs=1, space="PSUM") as sps,
            ):
                st = {}               # per-slot live tiles
                NR1 = 3 if R1 else 2
                s_ps = sps.tile([1, 2, BS], F32, tag="s")

                def phase_a(k):
                    pa = pnfp.tile([128, BB, KC, Npad], F16, tag="pnf")
                    dma(out=pa, in_=pnf_d[k])
                    pr = prfp.tile([128, BB, KC, Rpad], F16, tag="prf")
                    dma(out=pr, in_=prf_d[k])
                    st[("pnf", k)] = pa
                    st[("prf", k)] = pr

                def phase_a_compute(k):
                    pa = st.pop(("pnf", k))
                    pr = st.pop(("prf", k))
                    # bias adds; node first so its tanh can overlap rela adds
                    for i in range(BB):
                        b = k * BB + i
                        for c in range(KC):
                            nc.vector.tensor_scalar_add(
                                pa[:, i, c, :], pa[:, i, c, :],
                                nhT[:, c, b:b + 1])
                    nc.scalar.activation(pa, pa, AF.Tanh)
                    for i in range(BB):
                        b = k * BB + i
                        for c in range(KC):
                            nc.vector.tensor_scalar_add(
                                pr[:, i, c, :], pr[:, i, c, :],
                                rhT[:, c, b:b + 1])
                    nc.scalar.activation(pr, pr, AF.Tanh)
                    # scores: w_alpha stationary (1-col LDW), stream tanh;
                    # batch i's row lands at psum partition 32*i.
                    scb = scps.tile([128, Npad + Rpad], F32, tag="scb")
                    if k < 2:
                        nc.vector.memset(scb, 0.0)
                    for i in range(BB):
                        p0 = 32 * i
                        for c in range(KC):
                            nc.tensor.matmul(
                                scb[p0:p0 + 1, :Npad], w1c[:, c:c + 1],
                                pa[:, i, c, :],
                                start=(c == 0), stop=(c == KC - 1),
                                tile_position=(0, p0))
                        for c in range(KC):
                            nc.tensor.matmul(
                                scb[p0:p0 + 1, Npad:], w2c[:, c:c + 1],
                                pr[:, i, c, :],
                                start=(c == 0), stop=(c == KC - 1),
                                tile_position=(0, p0))
                    # exp on the whole block tile (dense rows)
                    em_d = emp.tile([128, Npad + Rpad], F16, tag="emd")
                    nc.scalar.activation(em_d, scb, AF.Exp)
                    # transpose em rows -> columns
                    et = etps.tile([128, NR1, 128], F16, tag="et")
                    nc.tensor.transpose(et[:Npad, 0, :], em_d[:, :Npad], ident)
                    nc.tensor.transpose(et[:R0, 1, :],
                                        em_d[:, Npad:Npad + R0], ident)
                    if R1:
                        nc.tensor.transpose(et[:R1, 2, :],
                                            em_d[:, Npad + R0:], ident)
                    # dense em columns for this block's batches
                    em_n = em2p.tile([Npad, BB], F16, tag="emn")
                    nc.vector.tensor_copy(em_n, et[:Npad, 0, 0:97:32])
                    em_r0 = em2p.tile([R0, BB], F16, tag="emr0")
                    nc.vector.tensor_copy(em_r0, et[:R0, 1, 0:97:32])
                    em_r1 = None
                    if R1:
                        em_r1 = em2p.tile([R1, BB], F16, tag="emr1")
                        nc.vector.tensor_copy(em_r1, et[:R1, 2, 0:97:32])
                    st[("em", k)] = (em_n, em_r0, em_r1)
                    # per-batch sums
                    b0 = k * BB
                    nc.tensor.matmul(s_ps[:, 0, b0:b0 + BB],
                                     ones_col[:Npad, :], em_n,
                                     start=True, stop=True)
                    nc.tensor.matmul(s_ps[:, 1, b0:b0 + BB],
                                     ones_col[:R0, :], em_r0,
                                     start=True, stop=(R1 == 0))
                    if R1:
                        nc.tensor.matmul(s_ps[:, 1, b0:b0 + BB],
                                         ones_col[:R1, :], em_r1,
                                         start=False, stop=True)

                def prefetch_values(v):
                    nf = nfp.tile([Npad, BB, KC, 128], F16, tag="nf")
                    dma(out=nf, in_=nf_d[v])
                    rf0 = rf0p.tile([R0, BB, KC, 128], F16, tag="rf0")
                    dma(out=rf0, in_=rf0_d[v])
                    st[("nf", v)] = nf
                    st[("rf0", v)] = rf0
                    if R1:
                        rf1 = rf1p.tile([R1, BB, KC, 128], F16, tag="rf1")
                        dma(out=rf1, in_=rf1_d[v])
                        st[("rf1", v)] = rf1

                def phase_c(g):
                    """weighted-sum matmuls for group g (values prefetched)."""
                    xp = xps.tile([128, 2, KC, G], F32, tag="xp")
                    for vj in range(NPJ):
                        blk = g * NPJ + vj
                        em_n, em_r0, em_r1 = st.pop(("em", blk))
                        nf = st.pop(("nf", blk))
                        rf0 = st.pop(("rf0", blk))
                        rf1 = st.pop(("rf1", blk)) if R1 else None
                        for i in range(BB):
                            jj = vj * BB + i
                            for c in range(KC):
                                nc.tensor.matmul(
                                    xp[:, 0, c, jj:jj + 1], nf[:, i, c, :],
                                    em_n[:, i:i + 1],
                                    start=True, stop=True)
                            for c in range(KC):
                                nc.tensor.matmul(
                                    xp[:, 1, c, jj:jj + 1], rf0[:, i, c, :],
                                    em_r0[:, i:i + 1],
                                    start=True, stop=(R1 == 0))
                                if R1:
                                    nc.tensor.matmul(
                                        xp[:, 1, c, jj:jj + 1],
                                        rf1[:, i, c, :],
                                        em_r1[:, i:i + 1],
                                        start=False, stop=True)
                    g0 = g * G
                    nc.vector.tensor_copy(XTn[:, :, g0:g0 + G], xp[:, 0])
                    nc.vector.tensor_copy(XTr[:, :, g0:g0 + G], xp[:, 1])

                PVLAG = 4  # value stream trails by 4 blocks
                for g in range(GROUPS):
                    for j in range(NPJ):
                        blk = g * NPJ + j
                        phase_a(blk)
                        if blk >= PVLAG:
                            prefetch_values(blk - PVLAG)
                        phase_a_compute(blk)
                    if g == 2:
                        dma(out=Wng_sb, in_=Wng_d)
                        dma(out=bng_row, in_=bngr_d)
                        dma(out=Wrg_sb, in_=Wrg_d)
                        dma(out=brg_row, in_=brgr_d)
                    if g == GROUPS - 1:
                        for v in range(NBLK - PVLAG, NBLK):
                            prefetch_values(v)
                    if g > 0:
                        phase_c(g - 1)
                phase_c(GROUPS - 1)
                nc.vector.tensor_copy(S_sb, s_ps)

            # ---- normalizers: rS columns [BS, 2] (node, rela) ----
            with (
                tc.tile_pool(name="glue", bufs=1) as gp,
                tc.tile_pool(name="glps", bufs=1, space="PSUM") as gps,
                tc.tile_pool(name="trps", bufs=1, space="PSUM") as tps_p,
            ):
                rS = gp.tile([1, 2, BS], F32, tag="rs")
                nc.vector.reciprocal(rS, S_sb)
                rS16 = gp.tile([1, 2, BS], F16, tag="rs16")
                nc.vector.tensor_copy(rS16, rS)
                Srow16 = gp.tile([1, 2, BS], F16, tag="srow")
                nc.vector.tensor_copy(Srow16, S_sb)
                rSc_ps = gps.tile([BS, 2], F32, tag="rscp")
                for k in range(2):
                    nc.tensor.matmul(rSc_ps[:, k:k + 1], rS16[:, k, :],
                                     ones_col[:1, :1], start=True, stop=True)
                rSc = gp.tile([BS, 2], F32, tag="rsc")
                nc.vector.tensor_copy(rSc, rSc_ps)

                # ---- GLU heads (interleaved: head-2's first matmul group
                # does not depend on node_res, so it overlaps head-1's
                # epilogue).  Bias rows enter the psum accumulation as
                # S-row x bias-row (exact after the 1/S scaling). ----
                def pmm(dst, lhs, W_sb, c0, srow=None, brow=None):
                    for hh in range(2):
                        for c in range(KC):
                            nc.tensor.matmul(dst[:, hh, :], lhs[:, c, :],
                                             W_sb[:, c0 + c, hh, :],
                                             start=(c == 0),
                                             stop=(c == KC - 1 and
                                                   srow is None))
                        if srow is not None:
                            nc.tensor.matmul(dst[:, hh, :], srow,
                                             brow[:, hh, :],
                                             start=False, stop=True)

                p1 = gps.tile([BS, 2, 512], F32, tag="p1")
                p2 = gps.tile([BS, 2, 512], F32, tag="p2")
                q1 = gps.tile([BS, 2, 512], F32, tag="q1")
                pmm(p1, XTn, Wng_sb, 0,                  # Xn @ Wng_top + Sn*b
                    srow=Srow16[:, 0, :], brow=bng_row)
                s1 = gp.tile([BS, 2, 512], F32, tag="ngs1")
                nc.vector.tensor_scalar_mul(s1, p1, rSc[:, 0:1])
                pmm(p2, XTr, Wng_sb, KC)                 # Xr @ Wng_bot
                pmm(q1, XTr, Wrg_sb, 0,                  # Xr @ Wrg_top + Sr*b
                    srow=Srow16[:, 1, :], brow=brg_row)
                t1 = gp.tile([BS, 2, 512], F32, tag="rgs1")
                nc.vector.tensor_scalar_mul(t1, q1, rSc[:, 1:2])
                nc.vector.scalar_tensor_tensor(
                    out=s1, in0=p2, scalar=rSc[:, 1:2],
                    in1=s1, op0=ALU.mult, op1=ALU.add)
                sig = gp.tile([BS, 512], F32, tag="ngsig")
                nc.scalar.activation(sig, s1[:, 1, :], AF.Sigmoid)
                nres = gp.tile([BS, 512], F32, tag="ngres")
                nc.vector.tensor_mul(nres, s1[:, 0, :], sig)
                dma(out=nres_d, in_=nres)

                # transpose node_res for the second head
                nres16 = gp.tile([BS, D], F16, tag="n16")
                nc.vector.tensor_copy(nres16, nres)
                nresT = gp.tile([128, KC, BS], F16, tag="nrt")
                for c in range(KC):
                    tps = tps_p.tile([128, BS], F16, tag="tps2")
                    nc.tensor.transpose(tps, nres16[:, c * 128:(c + 1) * 128],
                                        ident[:BS, :BS])
                    nc.vector.tensor_copy(nresT[:, c, :], tps)

                q2 = gps.tile([BS, 2, 512], F32, tag="p1")  # reuse p1's bank
                pmm(q2, nresT, Wrg_sb, KC)         # node_res @ Wrg_bot
                nc.vector.tensor_add(t1, t1, q2)
                sig2 = gp.tile([BS, 512], F32, tag="rgsig")
                nc.scalar.activation(sig2, t1[:, 1, :], AF.Sigmoid)
                rres = gp.tile([BS, 512], F32, tag="rgres")
                nc.vector.tensor_mul(rres, t1[:, 0, :], sig2)
                dma(out=rres_d, in_=rres)

    nc.compile()
    return nc


def _round32(x):
    return int(-(-int(x) // 32) * 32)


def _compact(feat, pfeat, mask, pad, pad_vec):
    """Gather active rows to the front, pad to `pad` rows.

    feat/pfeat: [B, L, D]; mask: [B, L] -> returns ([B,pad,D] f16 value rows
    zero-padded, [B,pad,D] f16 pre-activation rows pad_vec-padded).
    """
    order = np.argsort(mask == 0, axis=1, kind="stable")[:, :pad]
    valid = np.take_along_axis(mask, order, axis=1).astype(bool)[..., None]
    g = np.take_along_axis(feat, order[..., None], axis=1)
    gp = np.take_along_axis(pfeat, order[..., None], axis=1)
    fv = np.where(valid, g, np.float16(0.0))
    pv = np.where(valid, gp, pad_vec[None, None, :])
    return fv, pv


def make_in_maps(inputs, Npad, Rpad):
    """Shard + lay out full inputs into 8 per-core input dicts (host-side)."""
    f16 = np.float16
    f32 = np.float32
    R0 = min(Rpad, 128)
    R1 = Rpad - R0

    pnf = np.asarray(inputs["p_node_feats"], dtype=f16)
    nf = np.asarray(inputs["node_feats"], dtype=f16)
    prf = np.asarray(inputs["p_rela_feats"], dtype=f16)
    rf = np.asarray(inputs["rela_feats"], dtype=f16)
    h = np.asarray(inputs["h"], dtype=f16)
    am = np.asarray(inputs["att_masks"])
    rm = np.asarray(inputs["rela_masks"])
    w1 = np.asarray(inputs["w_alpha1"], dtype=f32)
    w2 = np.asarray(inputs["w_alpha2"], dtype=f32)

    padn = (-20.0 * np.sign(w1)).astype(f16)
    padr = (-20.0 * np.sign(w2)).astype(f16)

    nf_c, pnf_c = _compact(nf, pnf, am, Npad, padn)     # [B, Npad, D]
    rf_c, prf_c = _compact(rf, prf, rm, Rpad, padr)     # [B, Rpad, D]

    def shuf_p(x, L):  # [BS,L,D] -> [NBLK,128,BB,KC,L]  (d-partition)
        x = x.reshape(NBLK, BB, L, KC, 128)
        return np.ascontiguousarray(x.transpose(0, 4, 1, 3, 2))

    def shuf_v(x, lo, hi):  # [BS,L,D] -> [NBLK,hi-lo,BB,KC,128] (n-partition)
        x = x[:, lo:hi, :].reshape(NBLK, BB, hi - lo, KC, 128)
        return np.ascontiguousarray(x.transpose(0, 2, 1, 3, 4))

    def wcols(w):  # [D] -> [128, KC]
        return np.ascontiguousarray(w.astype(f16).reshape(KC, 128).T)

    Wn = np.ascontiguousarray(
        np.asarray(inputs["W_h2node"], dtype=f16).reshape(KC, 128, D)
        .transpose(1, 0, 2))
    Wr = np.ascontiguousarray(
        np.asarray(inputs["W_h2rela"], dtype=f16).reshape(KC, 128, D)
        .transpose(1, 0, 2))
    Wng = np.ascontiguousarray(
        np.asarray(inputs["W_ng"], dtype=f16).reshape(KC2, 128, 2, 512)
        .transpose(1, 0, 2, 3))
    Wrg = np.ascontiguousarray(
        np.asarray(inputs["W_rg"], dtype=f16).reshape(KC2, 128, 2, 512)
        .transpose(1, 0, 2, 3))

    shared = {
        "w_h2node": Wn, "w_h2rela": Wr,
        "b_h2node": np.ascontiguousarray(
            np.asarray(inputs["b_h2node"], dtype=f32).reshape(KC, 128).T),
        "b_h2rela": np.ascontiguousarray(
            np.asarray(inputs["b_h2rela"], dtype=f32).reshape(KC, 128).T),
        "w1c": wcols(w1),
        "w2c": wcols(w2),
        "w_ng": Wng, "w_rg": Wrg,
        "bias_ng": np.ascontiguousarray(
            np.asarray(inputs["b_ng"], dtype=f16).reshape(1, 2, 512)),
        "bias_rg": np.ascontiguousarray(
            np.asarray(inputs["b_rg"], dtype=f16).reshape(1, 2, 512)),
        "ident": np.eye(128, dtype=f16),
        "ones_col": np.ones((128, 1), dtype=f16),
    }
    in_maps = []
    for cix in range(NCORES):
        s = slice(cix * BS, (cix + 1) * BS)
        m = {
            "h": np.ascontiguousarray(h[s]),
            "pnf": shuf_p(pnf_c[s], Npad), "prf": shuf_p(prf_c[s], Rpad),
            "nf": shuf_v(nf_c[s], 0, Npad),
            "rf0": shuf_v(rf_c[s], 0, R0),
            **shared,
        }
        if R1:
            m["rf1"] = shuf_v(rf_c[s], R0, Rpad)
        in_maps.append(m)
    return in_maps


_NC_CACHE = {}
LAST_RESULTS = None  # BassKernelResults of the most recent kernel() call


def kernel(**inputs):
    global LAST_RESULTS
    am = np.asarray(inputs["att_masks"])
    rm = np.asarray(inputs["rela_masks"])
    Npad = min(N, _round32(am.sum(1).max()))
    Rpad = min(R, _round32(rm.sum(1).max()))
    key = (Npad, Rpad)
    if key not in _NC_CACHE:
        _NC_CACHE[key] = build_program(Npad, Rpad)
    nc = _NC_CACHE[key]
    in_maps = make_in_maps(inputs, Npad, Rpad)
    import os
    trace = os.environ.get("BASS_KERNEL_TRACE", "0") == "1"
    res = run_bass_kernel_spmd(nc, in_maps, core_ids=list(range(NCORES)),
                               trace=trace)
    LAST_RESULTS = res
    node_res = np.concatenate([r["node_res"] for r in res.results], axis=0)
    rela_res = np.concatenate([r["rela_res"] for r in res.results], axis=0)
    return node_res, rela_res


# revision 12
# speedup vs baseline: 1.2003x; 1.2003x over previous
"""Trainium2 Bass kernel for nn_Attention_60885456388891 (gnn_message_passing).

Computation (per batch b):
  node_h = h @ W_h2node + b_h2node
  score_n[n] = sum_d tanh(p_node_feats[b,n,d] + node_h[b,d]) * w_alpha1[d]
  node_w = renorm(softmax(score_n) * att_masks)
  node_res_ = sum_n node_w[n] * node_feats[b,n,:]
  (same for relations)
  node_res = glu(cat(node_res_, rela_res_) @ W_ng + b_ng)
  rela_res = glu(cat(rela_res_, node_res) @ W_rg + b_rg)

Strategy (v5): pure data-parallel over batch B=512 across 8 cores.

Key observation: softmax's denominator cancels against the mask renorm
(m*e^s / sum(m*e^s)), so masked nodes/relations are completely irrelevant
to the output.  The host COMPACTS each batch's active nodes/relations
(gather by mask) and pads to Npad/Rpad (runtime-derived from the masks,
rounded up to 32).  Pad positions get pnf = -20*sign(w_alpha) (=> score
-sum|w| => exp ~ e^-8, negligible) and value rows = 0, so no mask tensor
or mask arithmetic exists on device at all.  This cuts the streamed bytes
by ~40% on top of the f16 downcast (~39MB/core vs 56MB).

Per-core pipeline:
  - pnf/prf streamed d-on-partitions [128, BB, KC, Npad]; per-(b,c) DVE
    tensor_scalar_add of the nh/rh bias columns; one big ACT tanh per block.
  - scores: tanh chunk stationary [128d, Npad] x w_alpha chunk column ->
    em columns accumulate in PSUM **in column form** [Npad, G] -- exp (ACT)
    and the 1/sum reduction (ones-column matmul) happen in 128-partition
    form, and the em columns feed the value matmuls directly.
  - values as weights [K=Npad, M=128d] x em column -> X^T columns in the
    k-chunked layout the GLU matmuls consume.
  - GLU biases folded into the matmul: appending the sum-column S with
    rhs=bias_row gives (X@W + S*b) * (1/S) = X@W/S + b exactly.
"""

import numpy as np

import concourse.bass as bass
import concourse.bacc as bacc
import concourse.mybir as mybir
import concourse.tile as tile
from concourse.bass_utils import run_bass_kernel_spmd

# Problem dims (hardcoded per contract)
B, N, R, D = 512, 128, 256, 512
NCORES = 8
BS = B // NCORES          # 64 batches per core
BB = 4                    # batches per stream DMA block
NBLK = BS // BB           # 16 blocks
G = 16                    # batches per group (psum granularity)
GROUPS = BS // G          # 4 groups
NPJ = G // BB             # blocks per group
KC = D // 128             # 4 k-chunks of 128
KC2 = 2 * D // 128        # 8 k-chunks for the 1024-wide GLU matmuls

F32 = mybir.dt.float32
F16 = mybir.dt.float16
AF = mybir.ActivationFunctionType
ALU = mybir.AluOpType


def build_program(Npad, Rpad):
    R0 = min(Rpad, 128)
    R1 = Rpad - R0

    nc = bacc.Bacc("TRN2", target_bir_lowering=False, debug=False)

    def din(name, shape, dt=F16):
        return nc.dram_tensor(name, shape, dt, kind="ExternalInput").ap()

    h_d = din("h", [BS, D])
    pnf_d = din("pnf", [NBLK, 128, BB, KC, Npad])       # d-partition args
    prf_d = din("prf", [NBLK, 128, BB, KC, Rpad])
    nf_d = din("nf", [NBLK, Npad, BB, KC, 128])         # n-partition values
    rf0_d = din("rf0", [NBLK, R0, BB, KC, 128])
    rf1_d = din("rf1", [NBLK, R1, BB, KC, 128]) if R1 else None
    Wn_d = din("w_h2node", [128, KC, D])
    bn_d = din("b_h2node", [128, KC], F32)
    Wr_d = din("w_h2rela", [128, KC, D])
    br_d = din("b_h2rela", [128, KC], F32)
    w1_d = din("w1c", [128, KC])                        # w_alpha1 as columns
    w2_d = din("w2c", [128, KC])
    Wng_d = din("w_ng", [128, KC2, 2, 512])
    Wrg_d = din("w_rg", [128, KC2, 2, 512])
    bngr_d = din("bias_ng", [1, 2, 512])                # bias rows (f16)
    brgr_d = din("bias_rg", [1, 2, 512])
    id_d = din("ident", [128, 128])                     # f16 identity
    ones_d = din("ones_col", [128, 1])                  # f16 ones column

    nres_d = nc.dram_tensor("node_res", [BS, D], F32, kind="ExternalOutput").ap()
    rres_d = nc.dram_tensor("rela_res", [BS, D], F32, kind="ExternalOutput").ap()

    dma = nc.sync.dma_start

    with tile.TileContext(nc) as tc:
        with (
            tc.tile_pool(name="const", bufs=1) as cp,
        ):
            # ---- persistent constants ----
            ident = cp.tile([128, 128], F16)
            dma(out=ident, in_=id_d)
            ones_col = cp.tile([128, 1], F16)
            dma(out=ones_col, in_=ones_d)
            w1c = cp.tile([128, KC], F16)
            dma(out=w1c, in_=w1_d)
            w2c = cp.tile([128, KC], F16)
            dma(out=w2c, in_=w2_d)

            # persistent outputs of phase C / B
            XTn = cp.tile([128, KC, BS], F16, tag="xtn")   # unnormalized Xn^T
            XTr = cp.tile([128, KC, BS], F16, tag="xtr")
            S_sb = cp.tile([1, 2, BS], F32, tag="ssb")     # em column sums
            nhT = cp.tile([128, KC, BS], F32, tag="nht")   # bias columns
            rhT = cp.tile([128, KC, BS], F32, tag="rht")
            # GLU weights/bias rows (DMA'd during the main loop)
            Wng_sb = cp.tile([128, KC2, 2, 512], F16, tag="wng")
            Wrg_sb = cp.tile([128, KC2, 2, 512], F16, tag="wrg")
            bng_row = cp.tile([1, 2, 512], F16, tag="bngr")
            brg_row = cp.tile([1, 2, 512], F16, tag="brgr")

            # ---- prologue: nhT/rhT bias columns = (h @ W + b)^T, computed
            # directly in transposed chunk layout (no row round-trip) ----
            with (
                tc.tile_pool(name="prol", bufs=1) as pp,
                tc.tile_pool(name="prps", bufs=2, space="PSUM") as pps,
            ):
                h_sb = pp.tile([BS, D], F16, tag="h")
                dma(out=h_sb, in_=h_d)
                Wn_sb = pp.tile([128, KC, D], F16, tag="wn")
                dma(out=Wn_sb, in_=Wn_d)
                Wr_sb = pp.tile([128, KC, D], F16, tag="wr")
                dma(out=Wr_sb, in_=Wr_d)
                bn_sb = pp.tile([128, KC], F32, tag="bn")
                dma(out=bn_sb, in_=bn_d)
                br_sb = pp.tile([128, KC], F32, tag="br")
                dma(out=br_sb, in_=br_d)

                hT = pp.tile([128, KC, BS], F16, tag="ht")
                for c in range(KC):
                    tps = pps.tile([128, BS], F16, tag="tps")
                    nc.tensor.transpose(tps, h_sb[:, c * 128:(c + 1) * 128],
                                        ident[:BS, :BS])
                    nc.vector.tensor_copy(hT[:, c, :], tps)
                for W_sb, b_sb, dstT in ((Wn_sb, bn_sb, nhT),
                                         (Wr_sb, br_sb, rhT)):
                    for c in range(KC):
                        ps = pps.tile([128, BS], F32, tag="nhcps")
                        for k in range(KC):
                            nc.tensor.matmul(
                                ps, W_sb[:, k, c * 128:(c + 1) * 128],
                                hT[:, k, :],
                                start=(k == 0), stop=(k == KC - 1))
                        nc.vector.tensor_scalar_add(dstT[:, c, :], ps,
                                                    b_sb[:, c:c + 1])

            # ---- main loop: per-block software pipeline over 16 slots ----
            with (
                tc.tile_pool(name="pnfp", bufs=4) as pnfp,
                tc.tile_pool(name="prfp", bufs=4) as prfp,
                tc.tile_pool(name="nfp", bufs=4) as nfp,
                tc.tile_pool(name="rf0p", bufs=4) as rf0p,
                tc.tile_pool(name="rf1p", bufs=4) as rf1p,
                tc.tile_pool(name="emp", bufs=2) as emp,
                tc.tile_pool(name="scps", bufs=2, space="PSUM") as scps,
                tc.tile_pool(name="xps", bufs=2, space="PSUM") as xps,
                tc.tile_pool(name="sps", bufs=2, space="PSUM") as sps,
            ):
                st = {}               # per-slot live tiles

                def phase_a(k):
                    pa = pnfp.tile([128, BB, KC, Npad], F16, tag="pnf")
                    dma(out=pa, in_=pnf_d[k])
                    pr = prfp.tile([128, BB, KC, Rpad], F16, tag="prf")
                    dma(out=pr, in_=prf_d[k])
                    st[("pnf", k)] = pa
                    st[("prf", k)] = pr

                def phase_a_compute(k, scn, scr0, scr1, j):
                    """adds -> tanh -> score MMs at per-batch granularity so
                    DVE/ACT/PE pipeline within the block, not just across."""
                    pa = st.pop(("pnf", k))
                    pr = st.pop(("prf", k))
                    for i in range(BB):
                        b = k * BB + i
                        jj = j * BB + i
                        for c in range(KC):
                            nc.vector.tensor_scalar_add(
                                pa[:, i, c, :], pa[:, i, c, :],
                                nhT[:, c, b:b + 1])
                        nc.scalar.activation(pa[:, i], pa[:, i], AF.Tanh)
                        for c in range(KC):
                            nc.vector.tensor_scalar_add(
                                pr[:, i, c, :], pr[:, i, c, :],
                                rhT[:, c, b:b + 1])
                        nc.scalar.activation(pr[:, i], pr[:, i], AF.Tanh)
                        for c in range(KC):
                            nc.tensor.matmul(
                                scn[:, jj:jj + 1], pa[:, i, c, :],
                                w1c[:, c:c + 1],
                                start=(c == 0), stop=(c == KC - 1))
                        for c in range(KC):
                            nc.tensor.matmul(
                                scr0[:, jj:jj + 1], pr[:, i, c, :R0],
                                w2c[:, c:c + 1],
                                start=(c == 0), stop=(c == KC - 1))
                        if R1:
                            for c in range(KC):
                                nc.tensor.matmul(
                                    scr1[:, jj:jj + 1], pr[:, i, c, R0:],
                                    w2c[:, c:c + 1],
                                    start=(c == 0), stop=(c == KC - 1))

                def phase_b(g, scn, scr0, scr1):
                    """exp + column sums for group g (em in column form)."""
                    g0 = g * G
                    em_n = emp.tile([Npad, G], F16, tag="emn")
                    em_r0 = emp.tile([R0, G], F16, tag="emr0")
                    em_r1 = None
                    nc.scalar.activation(em_n, scn, AF.Exp)
                    nc.scalar.activation(em_r0, scr0, AF.Exp)
                    if R1:
                        em_r1 = emp.tile([R1, G], F16, tag="emr1")
                        nc.scalar.activation(em_r1, scr1, AF.Exp)
                    st[("em", g)] = (em_n, em_r0, em_r1)
                    s_ps = sps.tile([1, 2, G], F32, tag="s")
                    nc.tensor.matmul(s_ps[:, 0, :], ones_col[:Npad, :], em_n,
                                     start=True, stop=True)
                    nc.tensor.matmul(s_ps[:, 1, :], ones_col[:R0, :], em_r0,
                                     start=True, stop=(R1 == 0))
                    if R1:
                        nc.tensor.matmul(s_ps[:, 1, :], ones_col[:R1, :],
                                         em_r1, start=False, stop=True)
                    nc.vector.tensor_copy(S_sb[:, :, g0:g0 + G], s_ps)

                def prefetch_values(v):
                    nf = nfp.tile([Npad, BB, KC, 128], F16, tag="nf")
                    dma(out=nf, in_=nf_d[v])
                    rf0 = rf0p.tile([R0, BB, KC, 128], F16, tag="rf0")
                    dma(out=rf0, in_=rf0_d[v])
                    st[("nf", v)] = nf
                    st[("rf0", v)] = rf0
                    if R1:
                        rf1 = rf1p.tile([R1, BB, KC, 128], F16, tag="rf1")
                        dma(out=rf1, in_=rf1_d[v])
                        st[("rf1", v)] = rf1

                def phase_c_sub(g, vj):
                    """value matmuls for one 4-batch block of group g."""
                    em_n, em_r0, em_r1 = st[("em", g)]
                    xp = st[("xp", g)]
                    blk = g * NPJ + vj
                    nf = st.pop(("nf", blk))
                    rf0 = st.pop(("rf0", blk))
                    rf1 = st.pop(("rf1", blk)) if R1 else None
                    for i in range(BB):
                        jj = vj * BB + i
                        for c in range(KC):
                            nc.tensor.matmul(
                                xp[:, 0, c, jj:jj + 1], nf[:, i, c, :],
                                em_n[:, jj:jj + 1],
                                start=True, stop=True)
                        for c in range(KC):
                            nc.tensor.matmul(
                                xp[:, 1, c, jj:jj + 1], rf0[:, i, c, :],
                                em_r0[:, jj:jj + 1],
                                start=True, stop=(R1 == 0))
                            if R1:
                                nc.tensor.matmul(
                                    xp[:, 1, c, jj:jj + 1], rf1[:, i, c, :],
                                    em_r1[:, jj:jj + 1],
                                    start=False, stop=True)
                    if vj == NPJ - 1:
                        st.pop(("em", g))
                        st.pop(("xp", g))
                        g0 = g * G
                        nc.vector.tensor_copy(XTn[:, :, g0:g0 + G], xp[:, 0])
                        nc.vector.tensor_copy(XTr[:, :, g0:g0 + G], xp[:, 1])

                PVLAG = 2  # value blocks issue 2 slots before consumption
                for g in range(GROUPS):
                    sc = scps.tile([128, 3 if R1 else 2, G], F32, tag="sc")
                    scn = sc[:Npad, 0, :]
                    scr0 = sc[:, 1, :]
                    scr1 = sc[:R1, 2, :] if R1 else None
                    for j in range(NPJ):
                        blk = g * NPJ + j
                        phase_a(blk)
                        if blk >= PVLAG:
                            prefetch_values(blk - PVLAG)
                        phase_a_compute(blk, scn, scr0, scr1, j)
                        if g > 0:
                            phase_c_sub(g - 1, j)
                    if g == 2:
                        dma(out=Wng_sb, in_=Wng_d)
                        dma(out=bng_row, in_=bngr_d)
                        dma(out=Wrg_sb, in_=Wrg_d)
                        dma(out=brg_row, in_=brgr_d)
                    if g == GROUPS - 1:
                        for v in range(NBLK - PVLAG, NBLK):
                            prefetch_values(v)
                    phase_b(g, scn, scr0, scr1)
                    xpt = xps.tile([128, 2, KC, G], F32, tag="xp")
                    st[("xp", g)] = xpt
                for j in range(NPJ):
                    phase_c_sub(GROUPS - 1, j)

            # ---- normalizers: rS columns [BS, 2] (node, rela) ----
            with (
                tc.tile_pool(name="glue", bufs=1) as gp,
                tc.tile_pool(name="glps", bufs=1, space="PSUM") as gps,
                tc.tile_pool(name="trps", bufs=1, space="PSUM") as tps_p,
            ):
                rS = gp.tile([1, 2, BS], F32, tag="rs")
                nc.vector.reciprocal(rS, S_sb)
                rS16 = gp.tile([1, 2, BS], F16, tag="rs16")
                nc.vector.tensor_copy(rS16, rS)
                Srow16 = gp.tile([1, 2, BS], F16, tag="srow")
                nc.vector.tensor_copy(Srow16, S_sb)
                rSc_ps = gps.tile([BS, 2], F32, tag="rscp")
                for k in range(2):
                    nc.tensor.matmul(rSc_ps[:, k:k + 1], rS16[:, k, :],
                                     ones_col[:1, :1], start=True, stop=True)
                rSc = gp.tile([BS, 2], F32, tag="rsc")
                nc.vector.tensor_copy(rSc, rSc_ps)

                # ---- GLU heads (interleaved: head-2's first matmul group
                # does not depend on node_res, so it overlaps head-1's
                # epilogue).  Bias rows enter the psum accumulation as
                # S-row x bias-row (exact after the 1/S scaling). ----
                def pmm(dst, lhs, W_sb, c0, srow=None, brow=None):
                    for hh in range(2):
                        for c in range(KC):
                            nc.tensor.matmul(dst[:, hh, :], lhs[:, c, :],
                                             W_sb[:, c0 + c, hh, :],
                                             start=(c == 0),
                                             stop=(c == KC - 1 and
                                                   srow is None))
                        if srow is not None:
                            nc.tensor.matmul(dst[:, hh, :], srow,
                                             brow[:, hh, :],
                                             start=False, stop=True)

                p1 = gps.tile([BS, 2, 512], F32, tag="p1")
                p2 = gps.tile([BS, 2, 512], F32, tag="p2")
                q1 = gps.tile([BS, 2, 512], F32, tag="q1")
                pmm(p1, XTn, Wng_sb, 0,                  # Xn @ Wng_top + Sn*b
                    srow=Srow16[:, 0, :], brow=bng_row)
                s1 = gp.tile([BS, 2, 512], F32, tag="ngs1")
                nc.vector.tensor_scalar_mul(s1, p1, rSc[:, 0:1])
                pmm(p2, XTr, Wng_sb, KC)                 # Xr @ Wng_bot
                pmm(q1, XTr, Wrg_sb, 0,                  # Xr @ Wrg_top + Sr*b
                    srow=Srow16[:, 1, :], brow=brg_row)
                t1 = gp.tile([BS, 2, 512], F32, tag="rgs1")
                nc.vector.tensor_scalar_mul(t1, q1, rSc[:, 1:2])
                nc.vector.scalar_tensor_tensor(
                    out=s1, in0=p2, scalar=rSc[:, 1:2],
                    in1=s1, op0=ALU.mult, op1=ALU.add)
                sig = gp.tile([BS, 512], F32, tag="ngsig")
                nc.scalar.activation(sig, s1[:, 1, :], AF.Sigmoid)
                nres = gp.tile([BS, 512], F32, tag="ngres")
                nc.vector.tensor_mul(nres, s1[:, 0, :], sig)
                dma(out=nres_d, in_=nres)

                # transpose node_res for the second head
                nres16 = gp.tile([BS, D], F16, tag="n16")
                nc.vector.tensor_copy(nres16, nres)
                nresT = gp.tile([128, KC, BS], F16, tag="nrt")
                for c in range(KC):
                    tps = tps_p.tile([128, BS], F16, tag="tps2")
                    nc.tensor.transpose(tps, nres16[:, c * 128:(c + 1) * 128],
                                        ident[:BS, :BS])
                    nc.vector.tensor_copy(nresT[:, c, :], tps)

                q2 = gps.tile([BS, 2, 512], F32, tag="p1")  # reuse p1's bank
                pmm(q2, nresT, Wrg_sb, KC)         # node_res @ Wrg_bot
                nc.vector.tensor_add(t1, t1, q2)
                sig2 = gp.tile([BS, 512], F32, tag="rgsig")
                nc.scalar.activation(sig2, t1[:, 1, :], AF.Sigmoid)
                rres = gp.tile([BS, 512], F32, tag="rgres")
                nc.vector.tensor_mul(rres, t1[:, 0, :], sig2)
                dma(out=rres_d, in_=rres)

    nc.compile()
    return nc


def _round32(x):
    return int(-(-int(x) // 32) * 32)


def _compact(feat, pfeat, mask, pad, pad_vec):
    """Gather active rows to the front, pad to `pad` rows.

    feat/pfeat: [B, L, D]; mask: [B, L] -> returns ([B,pad,D] f16 value rows
    zero-padded, [B,pad,D] f16 pre-activation rows pad_vec-padded).
    """
    order = np.argsort(mask == 0, axis=1, kind="stable")[:, :pad]
    valid = np.take_along_axis(mask, order, axis=1).astype(bool)[..., None]
    g = np.take_along_axis(feat, order[..., None], axis=1)
    gp = np.take_along_axis(pfeat, order[..., None], axis=1)
    fv = np.where(valid, g, np.float16(0.0))
    pv = np.where(valid, gp, pad_vec[None, None, :])
    return fv, pv


def make_in_maps(inputs, Npad, Rpad):
    """Shard + lay out full inputs into 8 per-core input dicts (host-side)."""
    f16 = np.float16
    f32 = np.float32
    R0 = min(Rpad, 128)
    R1 = Rpad - R0

    pnf = np.asarray(inputs["p_node_feats"], dtype=f16)
    nf = np.asarray(inputs["node_feats"], dtype=f16)
    prf = np.asarray(inputs["p_rela_feats"], dtype=f16)
    rf = np.asarray(inputs["rela_feats"], dtype=f16)
    h = np.asarray(inputs["h"], dtype=f16)
    am = np.asarray(inputs["att_masks"])
    rm = np.asarray(inputs["rela_masks"])
    w1 = np.asarray(inputs["w_alpha1"], dtype=f32)
    w2 = np.asarray(inputs["w_alpha2"], dtype=f32)

    padn = (-20.0 * np.sign(w1)).astype(f16)
    padr = (-20.0 * np.sign(w2)).astype(f16)

    nf_c, pnf_c = _compact(nf, pnf, am, Npad, padn)     # [B, Npad, D]
    rf_c, prf_c = _compact(rf, prf, rm, Rpad, padr)     # [B, Rpad, D]

    def shuf_p(x, L):  # [BS,L,D] -> [NBLK,128,BB,KC,L]  (d-partition)
        x = x.reshape(NBLK, BB, L, KC, 128)
        return np.ascontiguousarray(x.transpose(0, 4, 1, 3, 2))

    def shuf_v(x, lo, hi):  # [BS,L,D] -> [NBLK,hi-lo,BB,KC,128] (n-partition)
        x = x[:, lo:hi, :].reshape(NBLK, BB, hi - lo, KC, 128)
        return np.ascontiguousarray(x.transpose(0, 2, 1, 3, 4))

    def wcols(w):  # [D] -> [128, KC]
        return np.ascontiguousarray(w.astype(f16).reshape(KC, 128).T)

    Wn = np.ascontiguousarray(
        np.asarray(inputs["W_h2node"], dtype=f16).reshape(KC, 128, D)
        .transpose(1, 0, 2))
    Wr = np.ascontiguousarray(
        np.asarray(inputs["W_h2rela"], dtype=f16).reshape(KC, 128, D)
        .transpose(1, 0, 2))
    Wng = np.ascontiguousarray(
        np.asarray(inputs["W_ng"], dtype=f16).reshape(KC2, 128, 2, 512)
        .transpose(1, 0, 2, 3))
    Wrg = np.ascontiguousarray(
        np.asarray(inputs["W_rg"], dtype=f16).reshape(KC2, 128, 2, 512)
        .transpose(1, 0, 2, 3))

    shared = {
        "w_h2node": Wn, "w_h2rela": Wr,
        "b_h2node": np.ascontiguousarray(
            np.asarray(inputs["b_h2node"], dtype=f32).reshape(KC, 128).T),
        "b_h2rela": np.ascontiguousarray(
            np.asarray(inputs["b_h2rela"], dtype=f32).reshape(KC, 128).T),
        "w1c": wcols(w1),
        "w2c": wcols(w2),
        "w_ng": Wng, "w_rg": Wrg,
        "bias_ng": np.ascontiguousarray(
            np.asarray(inputs["b_ng"], dtype=f16).reshape(1, 2, 512)),
        "bias_rg": np.ascontiguousarray(
            np.asarray(inputs["b_rg"], dtype=f16).reshape(1, 2, 512)),
        "ident": np.eye(128, dtype=f16),
        "ones_col": np.ones((128, 1), dtype=f16),
    }
    in_maps = []
    for cix in range(NCORES):
        s = slice(cix * BS, (cix + 1) * BS)
        m = {
            "h": np.ascontiguousarray(h[s]),
            "pnf": shuf_p(pnf_c[s], Npad), "prf": shuf_p(prf_c[s], Rpad),
            "nf": shuf_v(nf_c[s], 0, Npad),
            "rf0": shuf_v(rf_c[s], 0, R0),
            **shared,
        }
        if R1:
            m["rf1"] = shuf_v(rf_c[s], R0, Rpad)
        in_maps.append(m)
    return in_maps


_NC_CACHE = {}
LAST_RESULTS = None  # BassKernelResults of the most recent kernel() call


def kernel(**inputs):
    global LAST_RESULTS
    am = np.asarray(inputs["att_masks"])
    rm = np.asarray(inputs["rela_masks"])
    Npad = min(N, _round32(am.sum(1).max()))
    Rpad = min(R, _round32(rm.sum(1).max()))
    key = (Npad, Rpad)
    if key not in _NC_CACHE:
        _NC_CACHE[key] = build_program(Npad, Rpad)
    nc = _NC_CACHE[key]
    in_maps = make_in_maps(inputs, Npad, Rpad)
    import os
    trace = os.environ.get("BASS_KERNEL_TRACE", "0") == "1"
    res = run_bass_kernel_spmd(nc, in_maps, core_ids=list(range(NCORES)),
                               trace=trace)
    LAST_RESULTS = res
    node_res = np.concatenate([r["node_res"] for r in res.results], axis=0)
    rela_res = np.concatenate([r["rela_res"] for r in res.results], axis=0)
    return node_res, rela_res


# revision 21
# speedup vs baseline: 1.2410x; 1.0340x over previous
"""Trainium2 Bass kernel for nn_Attention_60885456388891 (gnn_message_passing).

Computation (per batch b):
  node_h = h @ W_h2node + b_h2node
  score_n[n] = sum_d tanh(p_node_feats[b,n,d] + node_h[b,d]) * w_alpha1[d]
  node_w = renorm(softmax(score_n) * att_masks)
  node_res_ = sum_n node_w[n] * node_feats[b,n,:]
  (same for relations)
  node_res = glu(cat(node_res_, rela_res_) @ W_ng + b_ng)
  rela_res = glu(cat(rela_res_, node_res) @ W_rg + b_rg)

Strategy (v5): pure data-parallel over batch B=512 across 8 cores.

Key observation: softmax's denominator cancels against the mask renorm
(m*e^s / sum(m*e^s)), so masked nodes/relations are completely irrelevant
to the output.  The host COMPACTS each batch's active nodes/relations
(gather by mask) and pads to Npad/Rpad (runtime-derived from the masks,
rounded up to 32).  Pad positions get pnf = -20*sign(w_alpha) (=> score
-sum|w| => exp ~ e^-8, negligible) and value rows = 0, so no mask tensor
or mask arithmetic exists on device at all.  This cuts the streamed bytes
by ~40% on top of the f16 downcast (~39MB/core vs 56MB).

Per-core pipeline:
  - pnf/prf streamed d-on-partitions [128, BB, KC, Npad]; per-(b,c) DVE
    tensor_scalar_add of the nh/rh bias columns; one big ACT tanh per block.
  - scores: tanh chunk stationary [128d, Npad] x w_alpha chunk column ->
    em columns accumulate in PSUM **in column form** [Npad, G] -- exp (ACT)
    and the 1/sum reduction (ones-column matmul) happen in 128-partition
    form, and the em columns feed the value matmuls directly.
  - values as weights [K=Npad, M=128d] x em column -> X^T columns in the
    k-chunked layout the GLU matmuls consume.
  - GLU biases folded into the matmul: appending the sum-column S with
    rhs=bias_row gives (X@W + S*b) * (1/S) = X@W/S + b exactly.
"""

import numpy as np

import concourse.bass as bass
import concourse.bacc as bacc
import concourse.mybir as mybir
import concourse.tile as tile
from concourse.bass_utils import run_bass_kernel_spmd

# Problem dims (hardcoded per contract)
B, N, R, D = 512, 128, 256, 512
NCORES = 8
BS = B // NCORES          # 64 batches per core
BB = 4                    # batches per stream DMA block
NBLK = BS // BB           # 16 blocks
G = 16                    # batches per group (psum granularity)
GROUPS = BS // G          # 4 groups
NPJ = G // BB             # blocks per group
KC = D // 128             # 4 k-chunks of 128
KC2 = 2 * D // 128        # 8 k-chunks for the 1024-wide GLU matmuls

F32 = mybir.dt.float32
F16 = mybir.dt.float16
AF = mybir.ActivationFunctionType
ALU = mybir.AluOpType


def build_program(Npad, Rpad):
    R0 = min(Rpad, 128)
    R1 = Rpad - R0
    # Stack the rela-overflow (R1) rows into partitions [Npad, Npad+R1) of
    # the node-value tile + em column + score psum, so their weight loads /
    # matmuls land in a disjoint 32-row-group of the PE array and run
    # concurrently with the node ones (tile_position packing).
    SK = R1 > 0 and Npad + R1 <= 128 and Npad % 32 == 0 and R1 <= 32
    NSTK = Npad + R1 if SK else Npad

    nc = bacc.Bacc("TRN2", target_bir_lowering=False, debug=False)

    def din(name, shape, dt=F16):
        return nc.dram_tensor(name, shape, dt, kind="ExternalInput").ap()

    h_d = din("h", [BS, D])
    pnf_d = din("pnf", [NBLK, 128, BB, KC, Npad])       # d-partition args
    prf_d = din("prf", [NBLK, 128, BB, KC, Rpad])
    nf_d = din("nf", [NBLK, NSTK, BB, KC, 128])         # n-partition values
    rf0_d = din("rf0", [NBLK, R0, BB, KC, 128])
    rf1_d = None
    if R1 and not SK:
        rf1_d = din("rf1", [NBLK, R1, BB, KC, 128])
    Wn_d = din("w_h2node", [128, KC, D])
    bn_d = din("b_h2node", [128, KC], F32)
    Wr_d = din("w_h2rela", [128, KC, D])
    br_d = din("b_h2rela", [128, KC], F32)
    w1_d = din("w1c", [128, KC])                        # w_alpha1 as columns
    w2_d = din("w2c", [128, KC])
    Wng_d = din("w_ng", [128, KC2, 2, 512])
    Wrg_d = din("w_rg", [128, KC2, 2, 512])
    bngr_d = din("bias_ng", [1, 2, 512])                # bias rows (f16)
    brgr_d = din("bias_rg", [1, 2, 512])
    id_d = din("ident", [128, 128])                     # f16 identity
    ones_d = din("ones_col", [128, 1])                  # f16 ones column

    nres_d = nc.dram_tensor("node_res", [BS, D], F32, kind="ExternalOutput").ap()
    rres_d = nc.dram_tensor("rela_res", [BS, D], F32, kind="ExternalOutput").ap()

    dma = nc.sync.dma_start

    with tile.TileContext(nc) as tc:
        with (
            tc.tile_pool(name="const", bufs=1) as cp,
        ):
            # ---- persistent constants ----
            ident = cp.tile([128, 128], F16)
            dma(out=ident, in_=id_d)
            ones_col = cp.tile([128, 1], F16)
            dma(out=ones_col, in_=ones_d)
            w1c = cp.tile([128, KC], F16)
            dma(out=w1c, in_=w1_d)
            w2c = cp.tile([128, KC], F16)
            dma(out=w2c, in_=w2_d)

            # persistent outputs of phase C / B
            XTn = cp.tile([128, KC, BS], F16, tag="xtn")   # unnormalized Xn^T
            XTr = cp.tile([128, KC, BS], F16, tag="xtr")
            S_sb = cp.tile([1, 2, BS], F32, tag="ssb")     # em column sums
            nhT = cp.tile([128, KC, BS], F32, tag="nht")   # bias columns
            rhT = cp.tile([128, KC, BS], F32, tag="rht")
            # GLU weights/bias rows (DMA'd during the main loop)
            Wng_sb = cp.tile([128, KC2, 2, 512], F16, tag="wng")
            Wrg_sb = cp.tile([128, KC2, 2, 512], F16, tag="wrg")
            bng_row = cp.tile([1, 2, 512], F16, tag="bngr")
            brg_row = cp.tile([1, 2, 512], F16, tag="brgr")

            # ---- prologue: nhT/rhT bias columns = (h @ W + b)^T, computed
            # directly in transposed chunk layout (no row round-trip) ----
            with (
                tc.tile_pool(name="prol", bufs=1) as pp,
                tc.tile_pool(name="prps", bufs=2, space="PSUM") as pps,
            ):
                h_sb = pp.tile([BS, D], F16, tag="h")
                dma(out=h_sb, in_=h_d)
                Wn_sb = pp.tile([128, KC, D], F16, tag="wn")
                dma(out=Wn_sb, in_=Wn_d)
                Wr_sb = pp.tile([128, KC, D], F16, tag="wr")
                dma(out=Wr_sb, in_=Wr_d)
                bn_sb = pp.tile([128, KC], F32, tag="bn")
                dma(out=bn_sb, in_=bn_d)
                br_sb = pp.tile([128, KC], F32, tag="br")
                dma(out=br_sb, in_=br_d)

                hT = pp.tile([128, KC, BS], F16, tag="ht")
                for c in range(KC):
                    tps = pps.tile([128, BS], F16, tag="tps")
                    nc.tensor.transpose(tps, h_sb[:, c * 128:(c + 1) * 128],
                                        ident[:BS, :BS])
                    nc.vector.tensor_copy(hT[:, c, :], tps)
                for W_sb, b_sb, dstT in ((Wn_sb, bn_sb, nhT),
                                         (Wr_sb, br_sb, rhT)):
                    for c in range(KC):
                        ps = pps.tile([128, BS], F32, tag="nhcps")
                        for k in range(KC):
                            nc.tensor.matmul(
                                ps, W_sb[:, k, c * 128:(c + 1) * 128],
                                hT[:, k, :],
                                start=(k == 0), stop=(k == KC - 1))
                        nc.vector.tensor_scalar_add(dstT[:, c, :], ps,
                                                    b_sb[:, c:c + 1])

            # ---- main loop: per-block software pipeline over 16 slots ----
            with (
                tc.tile_pool(name="pnfp", bufs=4) as pnfp,
                tc.tile_pool(name="prfp", bufs=4) as prfp,
                tc.tile_pool(name="nfp", bufs=4) as nfp,
                tc.tile_pool(name="rf0p", bufs=4) as rf0p,
                tc.tile_pool(name="rf1p", bufs=4) as rf1p,
                tc.tile_pool(name="emp", bufs=2) as emp,
                tc.tile_pool(name="scps", bufs=2, space="PSUM") as scps,
                tc.tile_pool(name="xps", bufs=2, space="PSUM") as xps,
                tc.tile_pool(name="sps", bufs=2, space="PSUM") as sps,
            ):
                st = {}               # per-slot live tiles

                def phase_a(k):
                    pa = pnfp.tile([128, BB, KC, Npad], F16, tag="pnf")
                    dma(out=pa, in_=pnf_d[k])
                    pr = prfp.tile([128, BB, KC, Rpad], F16, tag="prf")
                    dma(out=pr, in_=prf_d[k])
                    st[("pnf", k)] = pa
                    st[("prf", k)] = pr

                def phase_a_compute(k, scn, scr0, scr1, j):
                    """adds -> tanh -> score MMs at per-batch granularity so
                    DVE/ACT/PE pipeline within the block, not just across."""
                    pa = st.pop(("pnf", k))
                    pr = st.pop(("prf", k))
                    for i in range(BB):
                        b = k * BB + i
                        jj = j * BB + i
                        for c in range(KC):
                            nc.vector.tensor_scalar_add(
                                pa[:, i, c, :], pa[:, i, c, :],
                                nhT[:, c, b:b + 1])
                        nc.scalar.activation(pa[:, i], pa[:, i], AF.Tanh)
                        for c in range(KC):
                            nc.vector.tensor_scalar_add(
                                pr[:, i, c, :], pr[:, i, c, :],
                                rhT[:, c, b:b + 1])
                        nc.scalar.activation(pr[:, i], pr[:, i], AF.Tanh)
                        for c in range(KC):
                            nc.tensor.matmul(
                                scn[:, jj:jj + 1], pa[:, i, c, :],
                                w1c[:, c:c + 1],
                                start=(c == 0), stop=(c == KC - 1))
                        for c in range(KC):
                            nc.tensor.matmul(
                                scr0[:, jj:jj + 1], pr[:, i, c, :R0],
                                w2c[:, c:c + 1],
                                start=(c == 0), stop=(c == KC - 1))
                        if R1:
                            tp1 = (0, Npad) if SK else None
                            for c in range(KC):
                                nc.tensor.matmul(
                                    scr1[:, jj:jj + 1], pr[:, i, c, R0:],
                                    w2c[:, c:c + 1],
                                    start=(c == 0), stop=(c == KC - 1),
                                    tile_position=tp1)

                def phase_b(g, scn, scr0, scr1):
                    """exp + column sums for group g (em in column form)."""
                    g0 = g * G
                    em_n = emp.tile([NSTK, G], F16, tag="emn")
                    em_r0 = emp.tile([R0, G], F16, tag="emr0")
                    em_r1 = None
                    nc.scalar.activation(em_n[:Npad, :], scn, AF.Exp)
                    nc.scalar.activation(em_r0, scr0, AF.Exp)
                    if SK:
                        nc.scalar.activation(em_n[Npad:, :], scr1, AF.Exp)
                    elif R1:
                        em_r1 = emp.tile([R1, G], F16, tag="emr1")
                        nc.scalar.activation(em_r1, scr1, AF.Exp)
                    st[("em", g)] = (em_n, em_r0, em_r1)
                    s_ps = sps.tile([1, 2, G], F32, tag="s")
                    nc.tensor.matmul(s_ps[:, 0, :], ones_col[:Npad, :],
                                     em_n[:Npad, :], start=True, stop=True)
                    nc.tensor.matmul(s_ps[:, 1, :], ones_col[:R0, :], em_r0,
                                     start=True, stop=(R1 == 0))
                    if SK:
                        nc.tensor.matmul(s_ps[:, 1, :], ones_col[Npad:NSTK, :],
                                         em_n[Npad:, :], start=False,
                                         stop=True, tile_position=(Npad, 0))
                    elif R1:
                        nc.tensor.matmul(s_ps[:, 1, :], ones_col[:R1, :],
                                         em_r1, start=False, stop=True)
                    nc.vector.tensor_copy(S_sb[:, :, g0:g0 + G], s_ps)

                def prefetch_values(v):
                    nf = nfp.tile([NSTK, BB, KC, 128], F16, tag="nf")
                    dma(out=nf, in_=nf_d[v])
                    rf0 = rf0p.tile([R0, BB, KC, 128], F16, tag="rf0")
                    dma(out=rf0, in_=rf0_d[v])
                    st[("nf", v)] = nf
                    st[("rf0", v)] = rf0
                    if R1 and not SK:
                        rf1 = rf1p.tile([R1, BB, KC, 128], F16, tag="rf1")
                        dma(out=rf1, in_=rf1_d[v])
                        st[("rf1", v)] = rf1

                def phase_c_sub(g, vj):
                    """value matmuls for one 4-batch block of group g.

                    With SK, the rf1 weights/em sit in partitions
                    [Npad, Npad+R1) -- a disjoint PE row-group -- so their
                    LDWEIGHTS pull ahead of the in-flight nf matmuls.
                    """
                    em_n, em_r0, em_r1 = st[("em", g)]
                    xp = st[("xp", g)]
                    blk = g * NPJ + vj
                    nf = st.pop(("nf", blk))
                    rf0 = st.pop(("rf0", blk))
                    rf1 = st.pop(("rf1", blk)) if (R1 and not SK) else None
                    for i in range(BB):
                        jj = vj * BB + i
                        for c in range(KC):
                            nc.tensor.matmul(
                                xp[:, 0, c, jj:jj + 1], nf[:Npad, i, c, :],
                                em_n[:Npad, jj:jj + 1],
                                start=True, stop=True)
                            if SK:
                                nc.tensor.matmul(
                                    xp[:, 2, c, jj:jj + 1],
                                    nf[Npad:, i, c, :],
                                    em_n[Npad:, jj:jj + 1],
                                    start=True, stop=True,
                                    tile_position=(Npad, 0))
                        for c in range(KC):
                            nc.tensor.matmul(
                                xp[:, 1, c, jj:jj + 1], rf0[:, i, c, :],
                                em_r0[:, jj:jj + 1],
                                start=True, stop=(R1 == 0 or SK))
                            if R1 and not SK:
                                nc.tensor.matmul(
                                    xp[:, 1, c, jj:jj + 1], rf1[:, i, c, :],
                                    em_r1[:, jj:jj + 1],
                                    start=False, stop=True)
                    if vj == NPJ - 1:
                        st.pop(("em", g))
                        st.pop(("xp", g))
                        g0 = g * G
                        nc.vector.tensor_copy(XTn[:, :, g0:g0 + G], xp[:, 0])
                        if SK:
                            tmp = emp.tile([128, KC, G], F32, tag="xtmp")
                            nc.vector.tensor_copy(tmp, xp[:, 2])
                            nc.vector.tensor_add(XTr[:, :, g0:g0 + G],
                                                 xp[:, 1], tmp)
                        else:
                            nc.vector.tensor_copy(XTr[:, :, g0:g0 + G],
                                                  xp[:, 1])

                PVLAG = 2  # value blocks issue 2 slots before consumption
                for g in range(GROUPS):
                    sc = scps.tile([128, 3 if R1 else 2, G], F32, tag="sc")
                    scn = sc[:Npad, 0, :]
                    scr0 = sc[:, 1, :]
                    scr1 = None
                    if SK:
                        scr1 = sc[Npad:NSTK, 2, :]
                    elif R1:
                        scr1 = sc[:R1, 2, :]
                    for j in range(NPJ):
                        blk = g * NPJ + j
                        phase_a(blk)
                        if blk >= PVLAG:
                            prefetch_values(blk - PVLAG)
                        phase_a_compute(blk, scn, scr0, scr1, j)
                        if g > 0:
                            phase_c_sub(g - 1, j)
                    if g == 2:
                        dma(out=Wng_sb, in_=Wng_d)
                        dma(out=bng_row, in_=bngr_d)
                        dma(out=Wrg_sb, in_=Wrg_d)
                        dma(out=brg_row, in_=brgr_d)
                    if g == GROUPS - 1:
                        for v in range(NBLK - PVLAG, NBLK):
                            prefetch_values(v)
                    phase_b(g, scn, scr0, scr1)
                    xpt = xps.tile([128, 3 if SK else 2, KC, G], F32,
                                   tag="xp")
                    st[("xp", g)] = xpt
                for j in range(NPJ):
                    phase_c_sub(GROUPS - 1, j)

            # ---- normalizers: rS columns [BS, 2] (node, rela) ----
            with (
                tc.tile_pool(name="glue", bufs=1) as gp,
                tc.tile_pool(name="glps", bufs=1, space="PSUM") as gps,
                tc.tile_pool(name="trps", bufs=1, space="PSUM") as tps_p,
            ):
                rS = gp.tile([1, 2, BS], F32, tag="rs")
                nc.vector.reciprocal(rS, S_sb)
                rS16 = gp.tile([1, 2, BS], F16, tag="rs16")
                nc.vector.tensor_copy(rS16, rS)
                Srow16 = gp.tile([1, 2, BS], F16, tag="srow")
                nc.vector.tensor_copy(Srow16, S_sb)
                rSc_ps = gps.tile([BS, 2], F32, tag="rscp")
                for k in range(2):
                    nc.tensor.matmul(rSc_ps[:, k:k + 1], rS16[:, k, :],
                                     ones_col[:1, :1], start=True, stop=True)
                rSc = gp.tile([BS, 2], F32, tag="rsc")
                nc.vector.tensor_copy(rSc, rSc_ps)

                # ---- GLU heads (interleaved: head-2's first matmul group
                # does not depend on node_res, so it overlaps head-1's
                # epilogue).  Bias rows enter the psum accumulation as
                # S-row x bias-row (exact after the 1/S scaling). ----
                def pmm(dst, lhs, W_sb, c0, srow=None, brow=None):
                    for hh in range(2):
                        for c in range(KC):
                            nc.tensor.matmul(dst[:, hh, :], lhs[:, c, :],
                                             W_sb[:, c0 + c, hh, :],
                                             start=(c == 0),
                                             stop=(c == KC - 1 and
                                                   srow is None))
                        if srow is not None:
                            nc.tensor.matmul(dst[:, hh, :], srow,
                                             brow[:, hh, :],
                                             start=False, stop=True)

                p1 = gps.tile([BS, 2, 512], F32, tag="p1")
                p2 = gps.tile([BS, 2, 512], F32, tag="p2")
                q1 = gps.tile([BS, 2, 512], F32, tag="q1")
                pmm(p1, XTn, Wng_sb, 0,                  # Xn @ Wng_top + Sn*b
                    srow=Srow16[:, 0, :], brow=bng_row)
                s1 = gp.tile([BS, 2, 512], F32, tag="ngs1")
                nc.vector.tensor_scalar_mul(s1, p1, rSc[:, 0:1])
                pmm(p2, XTr, Wng_sb, KC)                 # Xr @ Wng_bot
                pmm(q1, XTr, Wrg_sb, 0,                  # Xr @ Wrg_top + Sr*b
                    srow=Srow16[:, 1, :], brow=brg_row)
                t1 = gp.tile([BS, 2, 512], F32, tag="rgs1")
                nc.vector.tensor_scalar_mul(t1, q1, rSc[:, 1:2])
                nc.vector.scalar_tensor_tensor(
                    out=s1, in0=p2, scalar=rSc[:, 1:2],
                    in1=s1, op0=ALU.mult, op1=ALU.add)
                sig = gp.tile([BS, 512], F32, tag="ngsig")
                nc.scalar.activation(sig, s1[:, 1, :], AF.Sigmoid)
                nres = gp.tile([BS, 512], F32, tag="ngres")
                nc.vector.tensor_mul(nres, s1[:, 0, :], sig)
                dma(out=nres_d, in_=nres)

                # transpose node_res for the second head
                nres16 = gp.tile([BS, D], F16, tag="n16")
                nc.vector.tensor_copy(nres16, nres)
                nresT = gp.tile([128, KC, BS], F16, tag="nrt")
                for c in range(KC):
                    tps = tps_p.tile([128, BS], F16, tag="tps2")
                    nc.tensor.transpose(tps, nres16[:, c * 128:(c + 1) * 128],
                                        ident[:BS, :BS])
                    nc.vector.tensor_copy(nresT[:, c, :], tps)

                q2 = gps.tile([BS, 2, 512], F32, tag="p1")  # reuse p1's bank
                pmm(q2, nresT, Wrg_sb, KC)         # node_res @ Wrg_bot
                nc.vector.tensor_add(t1, t1, q2)
                sig2 = gp.tile([BS, 512], F32, tag="rgsig")
                nc.scalar.activation(sig2, t1[:, 1, :], AF.Sigmoid)
                rres = gp.tile([BS, 512], F32, tag="rgres")
                nc.vector.tensor_mul(rres, t1[:, 0, :], sig2)
                dma(out=rres_d, in_=rres)

    nc.compile()
    return nc


def _round32(x):
    return int(-(-int(x) // 32) * 32)


def _compact(feat, pfeat, mask, pad, pad_vec):
    """Gather active rows to the front, pad to `pad` rows.

    feat/pfeat: [B, L, D]; mask: [B, L] -> returns ([B,pad,D] f16 value rows
    zero-padded, [B,pad,D] f16 pre-activation rows pad_vec-padded).
    """
    order = np.argsort(mask == 0, axis=1, kind="stable")[:, :pad]
    valid = np.take_along_axis(mask, order, axis=1).astype(bool)[..., None]
    g = np.take_along_axis(feat, order[..., None], axis=1)
    gp = np.take_along_axis(pfeat, order[..., None], axis=1)
    fv = np.where(valid, g, np.float16(0.0))
    pv = np.where(valid, gp, pad_vec[None, None, :])
    return fv, pv


def make_in_maps(inputs, Npad, Rpad):
    """Shard + lay out full inputs into 8 per-core input dicts (host-side)."""
    f16 = np.float16
    f32 = np.float32
    R0 = min(Rpad, 128)
    R1 = Rpad - R0

    pnf = np.asarray(inputs["p_node_feats"], dtype=f16)
    nf = np.asarray(inputs["node_feats"], dtype=f16)
    prf = np.asarray(inputs["p_rela_feats"], dtype=f16)
    rf = np.asarray(inputs["rela_feats"], dtype=f16)
    h = np.asarray(inputs["h"], dtype=f16)
    am = np.asarray(inputs["att_masks"])
    rm = np.asarray(inputs["rela_masks"])
    w1 = np.asarray(inputs["w_alpha1"], dtype=f32)
    w2 = np.asarray(inputs["w_alpha2"], dtype=f32)

    padn = (-20.0 * np.sign(w1)).astype(f16)
    padr = (-20.0 * np.sign(w2)).astype(f16)

    nf_c, pnf_c = _compact(nf, pnf, am, Npad, padn)     # [B, Npad, D]
    rf_c, prf_c = _compact(rf, prf, rm, Rpad, padr)     # [B, Rpad, D]
    SK = R1 > 0 and Npad + R1 <= 128 and Npad % 32 == 0 and R1 <= 32
    if SK:
        # stack the rela overflow rows under the node values (see kernel)
        nf_c = np.concatenate([nf_c, rf_c[:, R0:, :]], axis=1)

    def shuf_p(x, L):  # [BS,L,D] -> [NBLK,128,BB,KC,L]  (d-partition)
        x = x.reshape(NBLK, BB, L, KC, 128)
        return np.ascontiguousarray(x.transpose(0, 4, 1, 3, 2))

    def shuf_v(x, lo, hi):  # [BS,L,D] -> [NBLK,hi-lo,BB,KC,128] (n-partition)
        x = x[:, lo:hi, :].reshape(NBLK, BB, hi - lo, KC, 128)
        return np.ascontiguousarray(x.transpose(0, 2, 1, 3, 4))

    def wcols(w):  # [D] -> [128, KC]
        return np.ascontiguousarray(w.astype(f16).reshape(KC, 128).T)

    Wn = np.ascontiguousarray(
        np.asarray(inputs["W_h2node"], dtype=f16).reshape(KC, 128, D)
        .transpose(1, 0, 2))
    Wr = np.ascontiguousarray(
        np.asarray(inputs["W_h2rela"], dtype=f16).reshape(KC, 128, D)
        .transpose(1, 0, 2))
    Wng = np.ascontiguousarray(
        np.asarray(inputs["W_ng"], dtype=f16).reshape(KC2, 128, 2, 512)
        .transpose(1, 0, 2, 3))
    Wrg = np.ascontiguousarray(
        np.asarray(inputs["W_rg"], dtype=f16).reshape(KC2, 128, 2, 512)
        .transpose(1, 0, 2, 3))

    shared = {
        "w_h2node": Wn, "w_h2rela": Wr,
        "b_h2node": np.ascontiguousarray(
            np.asarray(inputs["b_h2node"], dtype=f32).reshape(KC, 128).T),
        "b_h2rela": np.ascontiguousarray(
            np.asarray(inputs["b_h2rela"], dtype=f32).reshape(KC, 128).T),
        "w1c": wcols(w1),
        "w2c": wcols(w2),
        "w_ng": Wng, "w_rg": Wrg,
        "bias_ng": np.ascontiguousarray(
            np.asarray(inputs["b_ng"], dtype=f16).reshape(1, 2, 512)),
        "bias_rg": np.ascontiguousarray(
            np.asarray(inputs["b_rg"], dtype=f16).reshape(1, 2, 512)),
        "ident": np.eye(128, dtype=f16),
        "ones_col": np.ones((128, 1), dtype=f16),
    }
    in_maps = []
    for cix in range(NCORES):
        s = slice(cix * BS, (cix + 1) * BS)
        m = {
            "h": np.ascontiguousarray(h[s]),
            "pnf": shuf_p(pnf_c[s], Npad), "prf": shuf_p(prf_c[s], Rpad),
            "nf": shuf_v(nf_c[s], 0, nf_c.shape[1]),
            "rf0": shuf_v(rf_c[s], 0, R0),
            **shared,
        }
        if R1 and not SK:
            m["rf1"] = shuf_v(rf_c[s], R0, Rpad)
        in_maps.append(m)
    return in_maps


_NC_CACHE = {}
LAST_RESULTS = None  # BassKernelResults of the most recent kernel() call


def kernel(**inputs):
    global LAST_RESULTS
    am = np.asarray(inputs["att_masks"])
    rm = np.asarray(inputs["rela_masks"])
    Npad = min(N, _round32(am.sum(1).max()))
    Rpad = min(R, _round32(rm.sum(1).max()))
    key = (Npad, Rpad)
    if key not in _NC_CACHE:
        _NC_CACHE[key] = build_program(Npad, Rpad)
    nc = _NC_CACHE[key]
    in_maps = make_in_maps(inputs, Npad, Rpad)
    import os
    trace = os.environ.get("BASS_KERNEL_TRACE", "0") == "1"
    res = run_bass_kernel_spmd(nc, in_maps, core_ids=list(range(NCORES)),
                               trace=trace)
    LAST_RESULTS = res
    node_res = np.concatenate([r["node_res"] for r in res.results], axis=0)
    rela_res = np.concatenate([r["rela_res"] for r in res.results], axis=0)
    return node_res, rela_res
